# revision 1
# baseline (speedup 1.0000x reference)
"""Canny NMS-detection TRN2 Bass kernel — full-input entry point.

Shards B=32 images across 8 NeuronCores (4 images/core, pure data
parallel), runs a Bass/Tile kernel per core, gathers full outputs.

Self-contained: hardcodes shapes (B,C,H,W = 32,3,512,512) and builds all
constants internally.
"""
import sys

for _p in ("/opt/trn_rl_repo", "/root/.axon_site/_ro/trn_rl_repo"):
    if _p not in sys.path:
        sys.path.append(_p)

import numpy as np
import concourse.bass as bass
import concourse.bacc as bacc
import concourse.tile as tile
import concourse.mybir as mybir
from concourse.bass_utils import run_bass_kernel_spmd

F32 = mybir.dt.float32
BF16 = mybir.dt.bfloat16
I8 = mybir.dt.int8
OP = mybir.AluOpType

A_G = float(np.exp(np.float32(-0.5)))
T_TH = float(np.float32(np.tan(22.5 * 3.14159 / 180.0)))
U_TH = float(np.float32(np.tan(67.5 * 3.14159 / 180.0)))
EPS = 1e-05
THRESHOLD = 3.0

N_CORES = 8
B, C, H, W = 32, 3, 512, 512
B_LOC = B // N_CORES

# ---------------------------------------------------------------- custom DVE ops
from concourse.dve_spec import Spec, Src0, Src1, C0, Zero, sq, select
from concourse.dve_ops import DveOp
import concourse.dve_ops as _dops

SQ_ADD_ANT = DveOp(
    "SQ_ADD_ANT",
    Spec(body=sq(Src0) + Src1,
         reference=lambda in0, in1, s0, s1, imm2: (in0 * in0 + in1).astype(np.float32)),
    subdim=False,
    uops_sha={"v3": "c25b2d28b4344100", "v4": "1d3e59af24fded96"},
)
NMS_KEEP_ANT = DveOp(
    "NMS_KEEP_ANT",
    Spec(body=select(Src1 < Src0, Src0, Zero),
         reference=lambda in0, in1, s0, s1, imm2: np.where(in1 < in0, in0, 0.0).astype(np.float32)),
    subdim=False,
    uops_sha={"v3": "d86f8416d0d7b042", "v4": "f70e64aee8639ca3"},
)
THRESH_KEEP_ANT = DveOp(
    "THRESH_KEEP_ANT",
    Spec(body=select(Src0 < C0, Zero, Src0),
         reference=lambda in0, in1, s0, s1, imm2: np.where(in0 < s0, 0.0, in0).astype(np.float32)),
    subdim=False,
    uops_sha={"v3": "492e392642627ae1", "v4": "a0c9cee97ab65b1d"},
)
_MY_OPS = [SQ_ADD_ANT, NMS_KEEP_ANT, THRESH_KEEP_ANT]


def _register_dve_ops():
    for op in _MY_OPS:
        if op.name in _dops._SUB_OPCODE_FOR_NAME:
            continue
        _dops.OPS.append(op)
        _dops._SUB_OPCODE_FOR_NAME[op.name] = (
            _dops._CUSTOM_DVE_ROW_BASE + len(_dops.OPS) - 1)
        _dops.CUSTOM_DVE_SPECS[op.name] = op.spec
    assert max(_dops._SUB_OPCODE_FOR_NAME.values()) < 0x20


# ---------------------------------------------------------------- band matrices
def band_matrices(TR=128):
    a = np.float32(A_G)

    def tri(cm1, c0, cp1):
        m = np.zeros((TR, TR), np.float32)
        for p in range(TR):
            m[p, p] = c0
            if p - 1 >= 0:
                m[p - 1, p] = cm1
            if p + 1 < TR:
                m[p + 1, p] = cp1
        return m

    def up(c):
        m = np.zeros((TR, TR), np.float32)
        m[TR - 1, 0] = c
        return m

    def dn(c):
        m = np.zeros((TR, TR), np.float32)
        m[0, TR - 1] = c
        return m

    return np.stack([
        tri(a, 1.0, a), up(a), dn(a),
        tri(1.0, 2.0, 1.0), up(1.0), dn(1.0),
        tri(1.0, 0.0, -1.0), up(1.0), dn(-1.0),
    ])


# ---------------------------------------------------------------- bass builder
def build(B_loc=B_LOC, H_=H, W_=W, TR=128):
    C_ = C
    NT = H_ // TR
    W = W_
    WP = W + 2
    _register_dve_ops()
    nc = bacc.Bacc(None)
    img = nc.declare_dram_parameter("img", [B_loc, C_, H_, W], F32, isOutput=False)
    bands = nc.declare_dram_parameter("bands", [9, TR, TR], F32, isOutput=False)
    o_blur = nc.declare_dram_parameter("blur", [B_loc, C_, H_, W], F32, isOutput=True)
    o_gm = nc.declare_dram_parameter("gm", [B_loc, H_, W], F32, isOutput=True)
    o_or = nc.declare_dram_parameter("orient", [B_loc, H_, W], F32, isOutput=True)
    o_thin = nc.declare_dram_parameter("thin", [B_loc, H_, W], F32, isOutput=True)
    o_thr = nc.declare_dram_parameter("thresh", [B_loc, H_, W], F32, isOutput=True)
    o_early = nc.declare_dram_parameter("early", [B_loc, H_, W], F32, isOutput=True)

    with tile.TileContext(nc) as tc:
        with (
            tc.tile_pool(name="const", bufs=1) as constp,
            tc.tile_pool(name="xin", bufs=3) as xinp,
            tc.tile_pool(name="hb", bufs=13) as hbp,
            tc.tile_pool(name="hxy", bufs=13) as hxyp,
            tc.tile_pool(name="hs", bufs=4) as hsp,
            tc.tile_pool(name="gmp", bufs=4) as gmp,
            tc.tile_pool(name="st", bufs=2) as stp,
            tc.tile_pool(name="nms", bufs=1) as nmsp,
            tc.tile_pool(name="ps", bufs=5, space="PSUM") as psp,
            tc.tile_pool(name="psg", bufs=3, space="PSUM") as psgp,
        ):
            bm = constp.tile([TR, 9 * TR], F32)
            eps_ap = constp.tile([TR, 1], F32, name="eps_ap")
            nc.vector.memset(eps_ap[:], EPS)
            c180 = constp.tile([TR, 1], F32, name="c180")
            nc.vector.memset(c180[:], 180.0)
            zrow = constp.tile([TR, WP], F32, name="zrow")
            nc.vector.memset(zrow[:], 0.0)
            for i in range(9):
                nc.sync.dma_start(bm[:, i * TR:(i + 1) * TR], bands[i])

            def W_g(i):
                return bm[:, i * TR:(i + 1) * TR]

            def vconv_mm(ps, base, rhs_tiles, r):
                parts = [(base, rhs_tiles[r])]
                if r > 0:
                    parts.append((base + 1, rhs_tiles[r - 1]))
                if r < NT - 1:
                    parts.append((base + 2, rhs_tiles[r + 1]))
                n = len(parts)
                for i, (wi, rhs) in enumerate(parts):
                    nc.tensor.matmul(ps[:], W_g(wi), rhs[:],
                                     start=(i == 0), stop=(i == n - 1))

            AF = mybir.ActivationFunctionType
            for b in range(B_loc):
                hb = {}
                hx = {}
                hy = {}
                hxs = [None] * NT
                hys = [None] * NT
                gm = [None] * NT
                for c in range(C_):
                    for r in range(NT):
                        xt = xinp.tile([TR, WP], F32, tag="xt", name="xt")
                        nc.vector.memset(xt[:, 0:1], 0.0)
                        nc.vector.memset(xt[:, W + 1:WP], 0.0)
                        nc.sync.dma_start(xt[:, 1:W + 1],
                                          img[b, c, r * TR:(r + 1) * TR, :])
                        t = stp.tile([TR, W], F32, tag="t_hb", name="t")
                        hbt = hbp.tile([TR, W], F32, tag="hb", name="hbt")
                        nc.vector.scalar_tensor_tensor(
                            t[:], xt[:, 0:W], A_G, xt[:, 1:W + 1], OP.mult, OP.add)
                        nc.vector.scalar_tensor_tensor(
                            hbt[:], xt[:, 2:WP], A_G, t[:], OP.mult, OP.add)
                        hb[(c, r)] = hbt

                for c in range(C_):
                    hbl = [hb[(c, r)] for r in range(NT)]
                    for r in range(NT):
                        ps = psp.tile([TR, W], F32, tag="ps", name="ps")
                        vconv_mm(ps, 0, hbl, r)
                        bs = stp.tile([TR, WP], F32, tag="bs", name="bs")
                        nc.vector.memset(bs[:, 0:1], 0.0)
                        nc.vector.memset(bs[:, W + 1:WP], 0.0)
                        nc.scalar.copy(bs[:, 1:W + 1], ps[:])
                        nc.sync.dma_start(o_blur[b, c, r * TR:(r + 1) * TR, :],
                                          bs[:, 1:W + 1])
                        hxt = hxyp.tile([TR, W], F32, tag="hx", name="hxt")
                        hyt = hxyp.tile([TR, W], F32, tag="hy", name="hyt")
                        s2t = stp.tile([TR, W], F32, tag="s2t", name="s2t")
                        nc.vector.tensor_tensor(
                            hxt[:], bs[:, 0:W], bs[:, 2:WP], OP.subtract)
                        nc.gpsimd.tensor_tensor(
                            s2t[:], bs[:, 0:W], bs[:, 2:WP], OP.add)
                        nc.vector.scalar_tensor_tensor(
                            hyt[:], ps[:], 2.0, s2t[:], OP.mult, OP.add)
                        hx[(c, r)] = hxt
                        hy[(c, r)] = hyt
                        if c == 0:
                            hxs[r] = hsp.tile([TR, W], F32, tag="hxs", name="hxs")
                            hys[r] = hsp.tile([TR, W], F32, tag="hys", name="hys")
                            nc.sync.dma_start(hxs[r][:], hxt[:])
                            nc.sync.dma_start(hys[r][:], hyt[:])
                        else:
                            nc.gpsimd.dma_start(hxs[r][:], hxt[:], accum_op=OP.add)
                            nc.gpsimd.dma_start(hys[r][:], hyt[:], accum_op=OP.add)

                for c in range(C_):
                    hxl = [hx[(c, r)] for r in range(NT)]
                    hyl = [hy[(c, r)] for r in range(NT)]
                    for r in range(NT):
                        gxp = psp.tile([TR, W], F32, tag="ps", name="gxp")
                        vconv_mm(gxp, 3, hxl, r)
                        t1 = stp.tile([TR, W], F32, tag="t1", name="t1")
                        nc.scalar.square(t1[:], gxp[:])
                        gyp = psp.tile([TR, W], F32, tag="ps", name="gyp")
                        vconv_mm(gyp, 6, hyl, r)
                        q = stp.tile([TR, W], F32, tag="q", name="q")
                        nc.vector._custom_dve(SQ_ADD_ANT, out=q[:], in0=gyp[:],
                                              in1=t1[:])
                        if c == 0:
                            gm[r] = gmp.tile([TR, WP], F32, tag="gm", name="gm")
                            nc.vector.memset(gm[r][:, 0:1], 0.0)
                            nc.vector.memset(gm[r][:, W + 1:WP], 0.0)
                            nc.scalar.activation(
                                gm[r][:, 1:W + 1], q[:], AF.Sqrt, bias=eps_ap[:])
                        else:
                            magt = stp.tile([TR, W], F32, tag="magt", name="magt")
                            nc.scalar.activation(
                                magt[:], q[:], AF.Sqrt, bias=eps_ap[:])
                            nc.gpsimd.dma_start(gm[r][:, 1:W + 1], magt[:],
                                                accum_op=OP.add)

                for r in range(NT):
                    gmi = gm[r][:, 1:W + 1]
                    GXp = psgp.tile([TR, W], F32, tag="psg", name="GXp")
                    vconv_mm(GXp, 3, hxs, r)
                    GYp = psgp.tile([TR, W], F32, tag="psg", name="GYp")
                    vconv_mm(GYp, 6, hys, r)
                    aX = nmsp.tile([TR, W], F32, tag="aX", name="aX")
                    aY = nmsp.tile([TR, W], F32, tag="aY", name="aY")
                    sx = nmsp.tile([TR, W], BF16, tag="sx", name="sx")
                    sy = nmsp.tile([TR, W], BF16, tag="sy", name="sy")
                    nc.scalar.activation(aX[:], GXp[:], AF.Abs)
                    nc.scalar.activation(aY[:], GYp[:], AF.Abs)
                    nc.scalar.sign(sx[:], GXp[:])
                    nc.scalar.sign(sy[:], GYp[:])
                    c1 = nmsp.tile([TR, W], BF16, tag="c1", name="c1")
                    c2 = nmsp.tile([TR, W], BF16, tag="c2", name="c2")
                    nc.vector.scalar_tensor_tensor(
                        c1[:], aX[:], T_TH, aY[:], OP.mult, OP.is_lt)
                    nc.vector.scalar_tensor_tensor(
                        c2[:], aX[:], U_TH, aY[:], OP.mult, OP.is_lt)
                    c2i = nmsp.tile([TR, W], I8, tag="c2i", name="c2i")
                    nc.scalar.copy(c2i[:], c2[:])
                    dm2 = nmsp.tile([TR, W], BF16, tag="dm2", name="dm2")
                    nc.vector.scalar_tensor_tensor(
                        dm2[:], c1[:], -2.0, c2[:], OP.add, OP.add)
                    rsx = nmsp.tile([TR, W], BF16, tag="rsx", name="rsx")
                    nc.vector.tensor_tensor(rsx[:], dm2[:], sx[:], OP.mult)
                    rsy = nmsp.tile([TR, W], BF16, tag="rsy", name="rsy")
                    nc.vector.scalar_tensor_tensor(
                        rsy[:], rsx[:], 2.0, sy[:], OP.add, OP.mult)
                    orient = nmsp.tile([TR, W], F32, tag="orient", name="orient")
                    nc.scalar.activation(orient[:], rsy[:], AF.Identity,
                                         bias=c180[:], scale=45.0)
                    nc.sync.dma_start(o_or[b, r * TR:(r + 1) * TR, :], orient[:])

                    sxy = nmsp.tile([TR, W], BF16, tag="sxy", name="sxy")
                    nc.vector.tensor_tensor(sxy[:], sx[:], sy[:], OP.mult)
                    sxyc1 = nmsp.tile([TR, W], BF16, tag="sxyc1", name="sxyc1")
                    nc.vector.tensor_tensor(sxyc1[:], sxy[:], c1[:], OP.mult)
                    mpos = nmsp.tile([TR, W], I8, tag="mpos", name="mpos")
                    mneg = nmsp.tile([TR, W], I8, tag="mneg", name="mneg")
                    nc.vector.tensor_scalar(mpos[:], sxyc1[:], 0.0, None, OP.is_gt)
                    nc.vector.tensor_scalar(mneg[:], sxyc1[:], 0.0, None, OP.is_lt)

                    gmN = nmsp.tile([TR, WP], F32, tag="gmN", bufs=2, name="gmN")
                    gmS = nmsp.tile([TR, WP], F32, tag="gmS", bufs=2, name="gmS")
                    nc.sync.dma_start(gmN[1:TR, :], gm[r][0:TR - 1, :])
                    if r > 0:
                        nc.sync.dma_start(gmN[0:1, :], gm[r - 1][TR - 1:TR, :])
                    else:
                        nc.sync.dma_start(gmN[0:1, :], zrow[0:1, :])
                    nc.sync.dma_start(gmS[0:TR - 1, :], gm[r][1:TR, :])
                    if r < NT - 1:
                        nc.sync.dma_start(gmS[TR - 1:TR, :], gm[r + 1][0:1, :])
                    else:
                        nc.sync.dma_start(gmS[TR - 1:TR, :], zrow[0:1, :])

                    pm0 = nmsp.tile([TR, W], F32, tag="pm0", name="pm0")
                    pm1 = nmsp.tile([TR, W], F32, tag="pm1", name="pm1")
                    pm2 = nmsp.tile([TR, W], F32, tag="pm2", name="pm2")
                    pm3 = nmsp.tile([TR, W], F32, tag="pm3", name="pm3")
                    nc.vector.tensor_tensor(
                        pm0[:], gm[r][:, 2:WP], gm[r][:, 0:W], OP.max)
                    nc.vector.tensor_tensor(
                        pm2[:], gmN[:, 1:W + 1], gmS[:, 1:W + 1], OP.max)
                    nc.vector.tensor_tensor(
                        pm1[:], gmS[:, 2:WP], gmN[:, 0:W], OP.max)
                    nc.vector.tensor_tensor(
                        pm3[:], gmS[:, 0:W], gmN[:, 2:WP], OP.max)
                    P = nmsp.tile([TR, W], F32, tag="P", name="P")
                    nc.vector.tensor_copy(P[:], pm0[:])
                    nc.vector.copy_predicated(P[:], mneg[:], pm3[:])
                    nc.vector.copy_predicated(P[:], mpos[:], pm1[:])
                    nc.vector.copy_predicated(P[:], c2i[:], pm2[:])
                    thin = nmsp.tile([TR, W], F32, tag="thin", name="thin")
                    nc.vector._custom_dve(NMS_KEEP_ANT, out=thin[:], in0=gmi,
                                          in1=P[:])
                    thr = nmsp.tile([TR, W], F32, tag="thr", name="thr")
                    nc.vector._custom_dve(THRESH_KEEP_ANT, out=thr[:],
                                          in0=thin[:], s0=THRESHOLD)
                    early = nmsp.tile([TR, W], F32, tag="early", name="early")
                    nc.vector._custom_dve(THRESH_KEEP_ANT, out=early[:],
                                          in0=gmi, s0=THRESHOLD)
                    rows = slice(r * TR, (r + 1) * TR)
                    nc.sync.dma_start(o_gm[b, rows, :], gmi)
                    nc.sync.dma_start(o_thin[b, rows, :], thin[:])
                    nc.sync.dma_start(o_thr[b, rows, :], thr[:])
                    nc.sync.dma_start(o_early[b, rows, :], early[:])
    nc.finalize()
    return nc


_NC_CACHE = {}


def _get_nc():
    if "nc" not in _NC_CACHE:
        _NC_CACHE["nc"] = build()
    return _NC_CACHE["nc"]


def kernel(img, gauss_h=None, gauss_v=None, sobel_h=None, sobel_v=None,
           dir_w=None):
    """Full-input kernel: img [32,3,512,512] fp32 -> 6-tuple matching
    reference.reference(). Filter-weight args are accepted and ignored
    (they are the fixed Canny constants, hardcoded in the band matrices)."""
    img = np.ascontiguousarray(np.asarray(img, dtype=np.float32))
    assert img.shape == (B, C, H, W), img.shape
    bands = band_matrices()
    nc = _get_nc()
    in_maps = [
        {"img": img[i * B_LOC:(i + 1) * B_LOC], "bands": bands}
        for i in range(N_CORES)
    ]
    last_err = None
    for _attempt in range(3):
        try:
            res = run_bass_kernel_spmd(nc, in_maps, list(range(N_CORES))).results
            break
        except Exception as e:  # transient device wedge: retry
            last_err = e
    else:
        raise last_err
    blur = np.concatenate([r["blur"] for r in res], axis=0)
    gm = np.concatenate([r["gm"] for r in res], axis=0)
    orient = np.concatenate([r["orient"] for r in res], axis=0)
    thin = np.concatenate([r["thin"] for r in res], axis=0)
    thr = np.concatenate([r["thresh"] for r in res], axis=0)
    early = np.concatenate([r["early"] for r in res], axis=0)
    return (
        blur.reshape(1, B, C, H, W),
        gm.reshape(B, 1, H, W),
        orient.reshape(B, 1, H, W),
        thin.reshape(B, 1, H, W),
        thr.reshape(B, 1, H, W),
        early.reshape(B, 1, H, W),
    )
